# revision 4
# baseline (speedup 1.0000x reference)
"""Multi-head self-attention (N=4, T=2048, D=1024, H=16) on 8 TRN2 NeuronCores.

Sharding: core c -> (batch n = c//2, head-group g = c%2 of 8 heads).

v3: one merged instruction stream built around the ScalarE exp stream.
  - bf16 matmul operands everywhere (1 cyc/row on PE; halves SBUF).
  - Packed transposed layouts xt_all[:, i*1024+d*128+c] = X[i*128+c, d*128+p]
    and w?T_all[:, b*1024+d*128+c] = W[b*128+c, d*128+p]: 4 PE transposes
    share one PSUM tile and evict as a single [128,512] copy.
  - Attention unit (0,0) starts right after X^T chunk 0 + K/Q block-0
    (~15us); all remaining prep (X^T stages, W^T blocks, K/Q blocks, V
    projection, Wo prep, first-half output projection) drains through a
    filler queue popped between attention j-steps under the exp stream.
  - slab is a ring of per-j [128,1024] tiles (ctx trails exp by ~16 steps).
  - ctx is q-split ([65,512] PSUM, 2 tiles per unit) and j-interleaved one
    unit behind S/exp; softmax normalization copies PSUM out first (frees
    the bank fast), then reciprocal + partition_broadcast from row 64.
  - First AllGather half fires mid-attention; its output-projection partial
    sums (+bias) are pre-accumulated into SBUF during late attention, so
    the tail is only: AG(1) + 4 matmuls/i + one add + batched store.
"""

from collections import deque
from contextlib import ExitStack

import numpy as np

import concourse.bass as bass
import concourse.mybir as mybir
import concourse.tile as tile
from concourse import bacc
from concourse.bass_utils import run_bass_kernel_spmd
from concourse.masks import make_identity

N, T, D, H, DH = 4, 2048, 1024, 16, 64
N_CORES = 8
G = 512            # per-core projection width (8 heads x 64)
HPC = 8            # heads per core
SCALE = 1.0 / 8.0  # 1/sqrt(DH)

f32 = mybir.dt.float32
f32r = mybir.dt.float32r
bf16 = mybir.dt.bfloat16
i32 = mybir.dt.int32

COMPUTE_DT = "bf16"

# global din-block order produced by the two half-AllGathers:
# half 0 carries heads 0-3 (blocks 0,1) + peer heads 8-11 (blocks 4,5)
CC_PERM = [[0, 1, 4, 5], [2, 3, 6, 7]]

TB = T // 128   # 16 token blocks
DB = D // 128   # 8 feature blocks
GB = G // 128   # 4 projected blocks

SLAB_RING = 22  # per-j slab tiles in flight (ctx trails exp by ~1 unit)


def build_nc(compute_dt: str = COMPUTE_DT, single_core: bool = False,
             reps: int = 0) -> bacc.Bacc:
    cdt = bf16

    nc = bacc.Bacc(
        "TRN2", target_bir_lowering=False, debug=False, num_devices=N_CORES
    )
    x_d = nc.dram_tensor("query", [T, D], f32, kind="ExternalInput").ap()
    m_d = nc.dram_tensor("mask", [T], i32, kind="ExternalInput").ap()
    wq_d = nc.dram_tensor("Wq", [G, D], f32, kind="ExternalInput").ap()
    wk_d = nc.dram_tensor("Wk", [G, D], f32, kind="ExternalInput").ap()
    wv_d = nc.dram_tensor("Wv", [G, D], f32, kind="ExternalInput").ap()
    wo_d = nc.dram_tensor("Wo", [G, D], f32, kind="ExternalInput").ap()
    bq_d = nc.dram_tensor("bq", [G], f32, kind="ExternalInput").ap()
    bk_d = nc.dram_tensor("bk", [G], f32, kind="ExternalInput").ap()
    bv_d = nc.dram_tensor("bv", [G], f32, kind="ExternalInput").ap()
    bo_d = nc.dram_tensor("bo", [G], f32, kind="ExternalInput").ap()
    out_d = nc.dram_tensor("out", [T, G], f32, kind="ExternalOutput").ap()

    with tile.TileContext(nc) as tc, ExitStack() as outer_ctx:
        if reps:
            outer_ctx.enter_context(tc.For_i(0, reps, 1))
        ctx = outer_ctx.enter_context(ExitStack())
        const = ctx.enter_context(tc.tile_pool(name="const", bufs=1))
        identity = const.tile([128, 128], f32)
        make_identity(nc, identity)
        bqk = const.tile([128, 2 * GB], f32, tag="bqk")
        bq_c, bk_c = bqk[:, 0:GB], bqk[:, GB:2 * GB]
        bvo = const.tile([128, 2 * G], f32, tag="bvo")
        bvb, bob = bvo[:, 0:G], bvo[:, G:2 * G]

        qpool = ctx.enter_context(tc.tile_pool(name="qpool", bufs=1))
        q_t = [qpool.tile([128, T], cdt, tag=f"q{i}", name=f"q{i}")
               for i in range(GB)]
        k_t = [qpool.tile([128, T], cdt, tag=f"k{i}", name=f"k{i}")
               for i in range(GB)]
        v_all = qpool.tile([128, TB * HPC * 65], bf16, tag="v_all")
        v_t = [v_all[:, i * HPC * 65:(i + 1) * HPC * 65] for i in range(TB)]
        woT_all = qpool.tile([128, GB * 1024], cdt, tag="woT")

        dram = ctx.enter_context(tc.tile_pool(name="dram", bufs=1, space="DRAM"))
        cc_in = dram.tile([G, T], bf16)
        cc_out = [dram.tile([G, T], bf16, name=f"cc_out{c}", tag=f"cc_out{c}")
                  for c in range(2)]
        # pair-3 (heads 6,7) goes through per-quarter tiles so the last
        # gather on the critical tail is only 128x512
        cc_p3in = [dram.tile([128, 512], bf16, name=f"ccp3i{q}",
                             tag=f"ccp3i{q}") for q in range(4)]
        cc_p3out = [dram.tile([256, 512], bf16, name=f"ccp3o{q}",
                              tag=f"ccp3o{q}") for q in range(4)]

        # long-lived pools (before the closeable prelude pools; LIFO stack)
        pp = ctx.enter_context(tc.tile_pool(name="pp", bufs=2, space="PSUM"))
        spp = ctx.enter_context(tc.tile_pool(name="spsum", bufs=2, space="PSUM"))
        cpp = ctx.enter_context(tc.tile_pool(name="cpsum", bufs=2, space="PSUM"))
        slabp = ctx.enter_context(tc.tile_pool(name="slab", bufs=SLAB_RING))
        zp = ctx.enter_context(tc.tile_pool(name="zbuf", bufs=2))
        csp = ctx.enter_context(tc.tile_pool(name="cstage", bufs=2))
        wsp = ctx.enter_context(tc.tile_pool(name="wstage", bufs=2))

        # prelude pools: released mid-attention
        late0 = ExitStack()
        xtp = late0.enter_context(tc.tile_pool(name="xtp", bufs=1))
        # d-major: xt_all[:, d*2048 + t] = X[t, d*128 + p] -- every matmul
        # operand slice is 2D-contiguous (strided rhs APs are slow on HW)
        xt_all = xtp.tile([128, TB * 1024], cdt, tag="xt_all")

        def xt_blk(i, d):
            return xt_all[:, d * 2048 + i * 128:d * 2048 + (i + 1) * 128]

        def xt_rhs(d, tch):
            return xt_all[:, d * 2048 + tch * 512:d * 2048 + (tch + 1) * 512]

        wpA = late0.enter_context(tc.tile_pool(name="wpA", bufs=1))
        wkT_all = wpA.tile([128, GB * 1024], cdt, tag="wkT")
        wqT_all = wpA.tile([128, GB * 1024], cdt, tag="wqT")
        wvT_all = wpA.tile([128, GB * 1024], cdt, tag="wvT")
        mkpool = late0.enter_context(tc.tile_pool(name="maskp", bufs=1))
        maskb = mkpool.tile([128, T], bf16, tag="maskb")

        stageE = ExitStack()
        sp = stageE.enter_context(tc.tile_pool(name="stage", bufs=2))
        mp = stageE.enter_context(tc.tile_pool(name="mload", bufs=1))

        xstage = {}

        def emit_stage_load(i):
            xs = sp.tile([128, D], f32, tag="stage", name="stage")
            xstage[i] = xs
            if i == 0:
                for ii in range(4):
                    nc.sync.dma_start(
                        xs[ii * 32:(ii + 1) * 32, :],
                        x_d[i * 128 + ii * 32:i * 128 + (ii + 1) * 32, :],
                    )
            else:
                nc.sync.dma_start(xs[:], x_d[i * 128:(i + 1) * 128, :])

        def emit_xt(i):
            xs = xstage.pop(i)
            for half in range(2):
                ps = pp.tile([128, 512], f32, tag="pp", name="pp")
                for k in range(4):
                    d = half * 4 + k
                    nc.tensor.transpose(
                        ps[:, k * 128:(k + 1) * 128],
                        xs[:, d * 128:(d + 1) * 128], identity[:]
                    )
                for k in range(4):
                    d = half * 4 + k
                    nc.any.tensor_copy(
                        xt_blk(i, d), ps[:, k * 128:(k + 1) * 128]
                    )

        def emit_wT(wdst_all, w_dram, b):
            ws = wsp.tile([128, D], f32, tag="wstage", name="wstage")
            nc.sync.dma_start(ws[:], w_dram[b * 128:(b + 1) * 128, :])
            for half in range(2):
                ps = pp.tile([128, 512], f32, tag="pp", name="pp")
                for k in range(4):
                    d = half * 4 + k
                    nc.tensor.transpose(
                        ps[:, k * 128:(k + 1) * 128],
                        ws[:, d * 128:(d + 1) * 128], identity[:]
                    )
                for k in range(4):
                    d = half * 4 + k
                    nc.any.tensor_copy(
                        wdst_all[:, d * 512 + b * 128:d * 512 + (b + 1) * 128],
                        ps[:, k * 128:(k + 1) * 128]
                    )

        def wT_lhs(w_all, d, b):
            return w_all[:, d * 512 + b * 128:d * 512 + (b + 1) * 128]

        def wT_rhs_alld(w_all, d):
            # all G dout columns for din chunk d (contiguous in d-major)
            return w_all[:, d * 512:(d + 1) * 512]

        # ---- DMA-queue-ordered prelude ----
        for i in range(4):
            emit_stage_load(i)
        # mask + biases
        m_i = mp.tile([1, T], i32)
        nc.sync.dma_start(m_i[:], m_d[None, :])
        m_f = mp.tile([1, T], bf16)
        nc.vector.tensor_copy(m_f[:], m_i[:])
        nc.gpsimd.partition_broadcast(maskb[:], m_f[:])
        nc.sync.dma_start(bq_c[:], bq_d.rearrange("(j p) -> p j", p=128))
        nc.sync.dma_start(bk_c[:], bk_d.rearrange("(j p) -> p j", p=128))
        bvo_r = mp.tile([1, 2 * G], f32, tag="bvor")
        nc.sync.dma_start(bvo_r[:, 0:G], bv_d[None, :])
        nc.sync.dma_start(bvo_r[:, G:2 * G], bo_d[None, :])
        nc.gpsimd.partition_broadcast(bvb[:], bvo_r[:, 0:G])
        nc.gpsimd.partition_broadcast(bob[:], bvo_r[:, G:2 * G])

        for i in range(4):
            emit_xt(i)
        emit_wT(wkT_all, wk_d, 0)
        emit_wT(wqT_all, wq_d, 0)

        def emit_qk_chunk(which, b, tch):
            wt_all = wkT_all if which == "k" else wqT_all
            dst = k_t[b] if which == "k" else q_t[b]
            ps = pp.tile([128, 512], f32, tag="pp", name="pp")
            for d in range(DB):
                nc.tensor.matmul(
                    ps[:],
                    wT_lhs(wt_all, d, b),
                    xt_rhs(d, tch),
                    start=(d == 0),
                    stop=(d == DB - 1),
                )
            if which == "k":
                nc.vector.tensor_scalar_add(
                    dst[:, tch * 512:(tch + 1) * 512], ps[:], bk_c[:, b:b + 1]
                )
            else:
                # masked-query columns of Q^T zeroed -> uniform softmax
                nc.vector.scalar_tensor_tensor(
                    dst[:, tch * 512:(tch + 1) * 512],
                    ps[:],
                    bq_c[:, b:b + 1],
                    maskb[:, tch * 512:(tch + 1) * 512],
                    op0=mybir.AluOpType.add,
                    op1=mybir.AluOpType.mult,
                )

        def emit_v_block(i):
            # V token-major [t, dout] with a ones column per head
            nc.gpsimd.memset(v_t[i][:], 1.0)
            ps = pp.tile([128, 512], f32, tag="pp", name="pp")
            for d in range(DB):
                nc.tensor.matmul(
                    ps[:],
                    xt_blk(i, d),
                    wT_rhs_alld(wvT_all, d),
                    start=(d == 0),
                    stop=(d == DB - 1),
                )
            # single strided eviction through a [128, 8, 64] view
            v3 = v_t[i].rearrange("p (h c) -> p h c", c=65)
            p3 = ps.rearrange("p (h c) -> p h c", c=64)
            b3 = bvb.rearrange("p (h c) -> p h c", c=64)
            nc.vector.tensor_tensor(
                v3[:, :, 0:64], p3[:, :, :], b3[:, :, :],
                op=mybir.AluOpType.add,
            )

        # prelude projections: K-b0 tch0, Q-b0 tch0 -> unit (0,0) can go
        emit_qk_chunk("k", 0, 0)
        emit_qk_chunk("q", 0, 0)

        # ---- filler queue ----
        fillers = deque()
        for tch in range(1, 4):
            for i in range(4 * tch, 4 * tch + 4):
                fillers.append(lambda i=i: (emit_stage_load(i), emit_xt(i)))
            fillers.append(lambda tch=tch: emit_qk_chunk("k", 0, tch))
        # positions: stages/K-b0 1..15
        for tch in range(1, 4):                              # 16-18
            fillers.append(lambda tch=tch: emit_qk_chunk("q", 0, tch))
        for b in range(GB):                                  # 19-22
            fillers.append(lambda b=b: emit_wT(wvT_all, wv_d, b))
        V_AT = 22
        for i in range(TB):                                  # 23-38
            fillers.append(lambda i=i: emit_v_block(i))
        for b in range(1, 4):                                # +10 each
            fillers.append(lambda b=b: emit_wT(wkT_all, wk_d, b))
            fillers.append(lambda b=b: emit_wT(wqT_all, wq_d, b))
            for tch in range(4):
                fillers.append(lambda b=b, tch=tch: emit_qk_chunk("k", b, tch))
            for tch in range(4):
                fillers.append(lambda b=b, tch=tch: emit_qk_chunk("q", b, tch))
        for b in range(GB):                                  # 69-72
            fillers.append(lambda b=b: emit_wT(woT_all, wo_d, b))

        # Q-b0 chunk tq needed at unit idx=tq (pair 0, quarter tq)
        QB0_GATE = {1: 16, 2: 17, 3: 18}
        KQ_DONE = {0: 18, 1: 48, 2: 58, 3: 68}
        # per-unit filler budgets, paced within the unit by Bresenham; the
        # pop_until gates below are correctness backstops
        POP_SCHED = {0: 22, 1: 16, 2: 5, 3: 5, 4: 3, 5: 3, 6: 2, 7: 2,
                     8: 3, 9: 3, 10: 2, 11: 2, 12: 4}
        n_popped = [0]

        def pop_fillers(k):
            for _ in range(k):
                if fillers:
                    n_popped[0] += 1
                    fillers.popleft()()

        def pop_until(target):
            while n_popped[0] < target and fillers:
                n_popped[0] += 1
                fillers.popleft()()

        # ---- attention stream: pair-units (head pair p, query quarter) ----
        # S matmuls for heads 2p/2p+1 are adjacent with disjoint PE row
        # strips (base partition 0 vs 64) -> they run concurrently in the
        # 128x128 array, halving effective S time on hardware.
        units = [(p, tq4) for p in range(HPC // 2) for tq4 in range(4)]
        slabs = {}        # (u) -> list of 16 per-j slab tiles
        cps_cur = {}

        def emit_s_j(u, j):
            p, tq4 = u
            t0 = tq4 * 512
            sl = slabp.tile([128, 1024], bf16, tag="slabj", name="slabj")
            slabs[u][j] = sl
            sps = spp.tile([128, 1024], f32, tag="sp", name="sp")
            for hh in range(2):
                hb = hh * 64
                nc.tensor.matmul(
                    sps[:, hh * 512:(hh + 1) * 512],
                    k_t[p][hb:hb + 64, j * 128:(j + 1) * 128],
                    q_t[p][hb:hb + 64, t0:t0 + 512],
                    start=True,
                    stop=True,
                )
            nc.scalar.activation(
                sl[:], sps[:], mybir.ActivationFunctionType.Exp, scale=SCALE,
            )

        def emit_ctx_j(u, j):
            p, tq4 = u
            sl = slabs[u][j]
            cq = cps_cur[u]
            for hh in range(2):
                h = 2 * p + hh
                nc.tensor.matmul(
                    cq[hh],
                    v_t[j][:, h * 65:h * 65 + 65],
                    sl[:, hh * 512:(hh + 1) * 512],
                    start=(j == 0),
                    stop=(j == TB - 1),
                )

        def emit_normalize(u):
            p, tq4 = u
            t0 = tq4 * 512
            del slabs[u]
            cq = cps_cur.pop(u)
            for hh in range(2):
                h = 2 * p + hh
                cps = cq[hh]
                cpy = zp.tile([65, 512], f32, tag="cpy", name="cpy")
                nc.vector.tensor_copy(cpy[:], cps[:])   # frees the PSUM bank
                # row 64 holds Z = sum_k exp; scale rows 0..63 by 1/Z.
                # partition_broadcast only reads partition 0 on real HW, so
                # hop the Z row down via DMA first.
                zr = zp.tile([1, 512], f32, tag="zr", name="zr", bufs=1)
                nc.sync.dma_start(zr[:], cpy[64:65, :])
                nc.vector.reciprocal(zr[:], zr[:])
                bct = zp.tile([64, 512], f32, tag="bc", name="bc", bufs=1)
                nc.gpsimd.partition_broadcast(bct[:], zr[:])
                cst = csp.tile([64, 512], bf16, tag="cst", name="cst")
                nc.vector.tensor_tensor(
                    cst[:], cpy[0:64, :], bct[:], op=mybir.AluOpType.mult
                )
                if p == 3:
                    nc.sync.dma_start(
                        cc_p3in[tq4][hh * 64:(hh + 1) * 64, :], cst[:]
                    )
                else:
                    nc.sync.dma_start(
                        cc_in[h * 64:(h + 1) * 64, t0:t0 + 512], cst[:]
                    )

        def emit_AG(c, r0, r1):
            # AllGather cc_in rows [r0:r1) -> cc_out[c]; peer rows at +nr
            nr = r1 - r0
            base = 2 * r0 - c * 512
            if single_core:
                nc.sync.dma_start(
                    cc_out[c][base:base + nr, :], cc_in[r0:r1, :]
                )
                nc.sync.dma_start(
                    cc_out[c][base + nr:base + 2 * nr, :], cc_in[r0:r1, :]
                )
            else:
                nc.gpsimd.collective_compute(
                    "AllGather",
                    mybir.AluOpType.bypass,
                    replica_groups=[[0, 1], [2, 3], [4, 5], [6, 7]],
                    ins=[cc_in[r0:r1, :].opt()],
                    outs=[cc_out[c][base:base + 2 * nr, :].opt()],
                )

        def emit_cf_loads(c, base, cf_tiles, cf_idx, nblk):
            for j in range(nblk):
                nc.sync.dma_start(
                    cf_tiles[cf_idx + j][:],
                    cc_out[c][base + j * 128:base + (j + 1) * 128, :]
                )

        def emit_AG_p3(q):
            # quarter gather for heads 6-7: [128,512] -> [256,512]
            if single_core:
                nc.sync.dma_start(cc_p3out[q][0:128, :], cc_p3in[q][:])
                nc.sync.dma_start(cc_p3out[q][128:256, :], cc_p3in[q][:])
            else:
                nc.gpsimd.collective_compute(
                    "AllGather",
                    mybir.AluOpType.bypass,
                    replica_groups=[[0, 1], [2, 3], [4, 5], [6, 7]],
                    ins=[cc_p3in[q][:].opt()],
                    outs=[cc_p3out[q][:].opt()],
                )
            nc.sync.dma_start(
                cf1[2][:, q * 512:(q + 1) * 512], cc_p3out[q][0:128, :]
            )
            nc.sync.dma_start(
                cf1[3][:, q * 512:(q + 1) * 512], cc_p3out[q][128:256, :]
            )

        late1 = ExitStack()
        cf0 = None
        partials = [None] * TB
        tail_fillers = deque()

        def emit_c0_part(i):
            # pre-accumulate first-half output projection (+ bias) into SBUF
            ps = pp.tile([128, 512], f32, tag="pp", name="pp")
            for j in range(GB):
                nc.tensor.matmul(
                    ps[:],
                    cf0[j][:, i * 128:(i + 1) * 128],
                    wT_rhs_alld(woT_all, CC_PERM[0][j]),
                    start=(j == 0), stop=(j == GB - 1),
                )
            pt = partp.tile([128, G], f32, tag=f"part{i}", name=f"part{i}")
            partials[i] = pt
            nc.vector.tensor_tensor(pt[:], ps[:], bob[:],
                                    op=mybir.AluOpType.add)

        def emit_c1a_part(i):
            # fold the first two cf1 blocks (from the heads-4/5 gather) into
            # the partial as well; the tail then only needs blocks 2/3
            ps = pp.tile([128, 512], f32, tag="pp", name="pp")
            for j in range(2):
                nc.tensor.matmul(
                    ps[:],
                    cf1[j][:, i * 128:(i + 1) * 128],
                    wT_rhs_alld(woT_all, CC1_MAP[j]),
                    start=(j == 0), stop=(j == 1),
                )
            nc.vector.tensor_tensor(partials[i][:], partials[i][:], ps[:],
                                    op=mybir.AluOpType.add)

        def emit_finish(i):
            # final two matmuls (cf1 blocks 2/3) + partial add + store
            ps = pp.tile([128, 512], f32, tag="pp", name="pp")
            for j in range(2, GB):
                nc.tensor.matmul(
                    ps[:],
                    cf1[j][:, i * 128:(i + 1) * 128],
                    wT_rhs_alld(woT_all, CC1_MAP[j]),
                    start=(j == 2), stop=(j == GB - 1),
                )
            osl = oslp.tile([128, G], f32, tag="osl", name="osl")
            nc.vector.tensor_tensor(osl[:], ps[:], partials[i][:],
                                    op=mybir.AluOpType.add)
            nc.sync.dma_start(out_d[i * 128:(i + 1) * 128, :], osl[:])

        cf1 = None
        CC1_MAP = [2, 6, 3, 7]   # cf1 block -> global din block (quarter AGs)
        for idx, u in enumerate(units):
            if idx == 9:
                # heads 0-3 all normalized (units 0-7): fire first gather;
                # SBUF-side cf0 loads wait until the prelude pools close
                emit_AG(0, 0, 256)
            if idx == 12:
                pop_until(68)    # everything reading xt/wkT/wqT/wvT/maskb
                stageE.close()
                late0.close()
                cf0p = late1.enter_context(tc.tile_pool(name="cf0", bufs=1))
                cf0 = [cf0p.tile([128, T], bf16, tag=f"cf0{j}",
                                 name=f"cf0{j}") for j in range(GB)]
                cf1p = late1.enter_context(tc.tile_pool(name="cf1", bufs=1))
                cf1 = [cf1p.tile([128, T], bf16, tag=f"cf1{j}",
                                 name=f"cf1{j}") for j in range(GB)]
                partp = late1.enter_context(
                    tc.tile_pool(name="partp", bufs=1))
                oslp = late1.enter_context(tc.tile_pool(name="oslp", bufs=3))
                emit_cf_loads(0, 0, cf0, 0, 4)
                for i in range(TB):
                    tail_fillers.append(lambda i=i: emit_c0_part(i))
            if idx == 13:
                # first quarter of the second gather (heads 4-5)
                emit_AG(1, 256, 384)
                emit_cf_loads(1, 0, cf1, 0, 2)
                for i in range(TB):
                    tail_fillers.append(lambda i=i: emit_c1a_part(i))
                for i in range(8):
                    tail_fillers.append(lambda i=i: emit_finish(i))
            if idx == 14:
                emit_AG_p3(0)    # (3,0) normalized at end of idx 13
            if idx == 15:
                emit_AG_p3(1)

            slabs[u] = [None] * TB
            if idx > 0:
                cps_cur[units[idx - 1]] = [
                    cpp.tile([65, 512], f32, tag="cp", name="cp")
                    for _ in range(2)
                ]
                pop_until(KQ_DONE[u[0]])
                if idx in QB0_GATE:
                    pop_until(QB0_GATE[idx])
            if idx == 14:
                pop_until(10 ** 9)   # woT fillers before c0 parts
            budget = POP_SCHED.get(idx, 0)
            done_before = 0
            for j in range(TB):
                if idx == 0 and j % 4 == 0 and j > 0:
                    pop_until(5 * (j // 4))
                emit_s_j(u, j)
                if idx > 0:
                    if idx == 1:
                        pop_until(V_AT + j + 1)   # v_t[j] producer
                    emit_ctx_j(units[idx - 1], j)
                want = (j + 1) * budget // TB
                if fillers and want > done_before:
                    pop_fillers(want - done_before)
                    done_before = want
                elif idx == 14 and tail_fillers:
                    tail_fillers.popleft()()   # c0 + c1a parts
                    if tail_fillers:
                        tail_fillers.popleft()()
                elif idx == 15 and tail_fillers and j % 2 == 0:
                    tail_fillers.popleft()()   # finish(0..7)
            if idx > 0:
                emit_normalize(units[idx - 1])

        # ---- drain: ctx of last unit + remaining tail fillers ----
        u15 = units[-1]
        cps_cur[u15] = [cpp.tile([65, 512], f32, tag="cp", name="cp")
                        for _ in range(2)]
        emit_AG_p3(2)    # (3,2) normalized at end of idx 15
        for j in range(TB):
            emit_ctx_j(u15, j)
            if tail_fillers:
                tail_fillers.popleft()()
        emit_normalize(u15)
        while tail_fillers:
            tail_fillers.popleft()()

        # ---- tail: last gather quarter (heads 6-7, queries 1536+);
        # finishes 8-11 (quarter 2, already gathered) run under the AG wait
        emit_AG_p3(3)
        for i in range(8, TB):
            emit_finish(i)
        late1.close()

    nc.compile()
    return nc


def shard_inputs(query, mask, Wq, bq, Wk, bk, Wv, bv, Wo, bo):
    in_maps = []
    for c in range(N_CORES):
        n, g = c // 2, c % 2
        sl = slice(g * G, (g + 1) * G)
        in_maps.append(
            {
                "query": np.ascontiguousarray(query[n], dtype=np.float32),
                "mask": np.ascontiguousarray(mask[n], dtype=np.int32),
                "Wq": np.ascontiguousarray(Wq[sl], dtype=np.float32),
                "Wk": np.ascontiguousarray(Wk[sl], dtype=np.float32),
                "Wv": np.ascontiguousarray(Wv[sl], dtype=np.float32),
                "Wo": np.ascontiguousarray(Wo[sl], dtype=np.float32),
                "bq": np.ascontiguousarray(bq[sl], dtype=np.float32),
                "bk": np.ascontiguousarray(bk[sl], dtype=np.float32),
                "bv": np.ascontiguousarray(bv[sl], dtype=np.float32),
                "bo": np.ascontiguousarray(bo[sl], dtype=np.float32),
            }
        )
    return in_maps


def gather_outputs(results):
    out = np.empty((N, T, D), np.float32)
    for c in range(N_CORES):
        n, g = c // 2, c % 2
        out[n][:, g * G:(g + 1) * G] = results[c]["out"]
    return out


def kernel(query, mask, Wq, bq, Wk, bk, Wv, bv, Wo, bo):
    # the neuron compile cache keys on the outer HLO only (the bass BIR is
    # in backend_config), so stale entries from other kernels collide --
    # key the cache dir by this file's content
    import os
    import hashlib
    h = hashlib.sha256(open(__file__, "rb").read()).hexdigest()[:16]
    os.environ["NEURON_COMPILE_CACHE_URL"] = f"/tmp/neuron-cache-{h}"
    in_maps = shard_inputs(query, mask, Wq, bq, Wk, bk, Wv, bv, Wo, bo)
    nc = build_nc()
    res = run_bass_kernel_spmd(nc, in_maps, list(range(N_CORES)))
    return gather_outputs(res.results)
